# revision 27
# baseline (speedup 1.0000x reference)
"""AdaptiveNeuromorphicNetwork Trainium2 kernel (8 NeuronCores, SPMD).

Sharding: output neurons H=2048 split 256/core (H-shard) -> the LIF scan,
spike-rate mean (over batch) and homeostatic threshold update are fully local
per core; zero collectives. input_spikes are replicated (each core streams all
of them through the TensorEngine against its weight column shard).

Matmul scheme ("fp16dc"): single fp16-weight pass with the MOVING operand
being the fp8 spikes directly (0/1 exact in fp8; cost model keys cycles/row
on the moving dtype, so this is the same 1 cyc/row as fp16 moving but needs
no fp16 spike copies/casts at all). The dropped fp8 residual plane is
compensated by a host-computed DC correction: the time-mean residual current
c[b,h] = -sum_i (fp16(w)-w)[i,h] * mean_t s[b,i,t], which the double-EMA
dynamics would otherwise amplify ~4x over the white part. c is injected into
the same PSUM accumulation group as a 64-row "rider" matmul (bf16 weights =
c per batch row, fp8 moving identity-per-batch pattern). CPU-validated rel
err 0.0165 (gate 2e-2); fp16-subnormal weights are zeroed host-side so the
host-side E matches the device bit-exactly.

Optional RESID_K adds a time-averaged fp8-DoubleRow residual plane over
packed spike groups (s_t+...+s_{t+K-1})/K — K=2 halves the white-noise
variance for 0.25 of a pass (est rel err 0.0117, +27us) — off by default.

Per-core pipeline over 9 time-chunks: DMA fp8 spikes -> matmul into PSUM
(16 k-tiles + rider) -> evac PSUM->SBUF (scalar engine) -> sequential LIF
scan (vector engine custom fused DVE ops; threshold EMA chain on gpsimd) ->
spikes accumulate in SBUF (bf16) -> chunked DMA out.
"""
import numpy as np

import concourse.bass as bass
import concourse.tile as tile
from concourse import bacc, mybir
from concourse.bass_utils import run_bass_kernel_spmd

B, I, H, T = 64, 2048, 2048, 128
NCORES = 8
HL = H // NCORES            # 256 output neurons per core
KT = I // 128               # 16 contraction tiles
# tapered at both ends: small first chunks let the PE start before the bulk
# spike DMA lands; small last chunks keep the serial LIF-scan tail short
CHUNKS = [2, 4, 8, 16, 16, 16, 16, 16, 16, 10, 4, 2, 2]
NCH = len(CHUNKS)
assert sum(CHUNKS) == T
DT = 0.001
TCS = sorted(set(CHUNKS))    # distinct chunk lengths (rider pattern per TC)


def _blocks(n):
    """Chop n columns into PSUM-bank-aligned matmul blocks (<=512 f32)."""
    out = []
    o = 0
    while o < n:
        b = min(512, n - o)
        out.append((o, b))
        o += b
    return out

RIDER = True        # inject DC correction via 64-row rider matmul
RESID_K = 0         # 0=off; 2/4 = K-step-averaged fp8 residual plane
TRACE = False
TRACE_KW = {}

_F32 = mybir.dt.float32
_ALU = mybir.AluOpType

# ---- custom fused DVE ops for the LIF step ----
import operator as _op

import concourse.dve_ops as _dve_ops
from concourse.dve_ops import DveOp as _DveOp
from concourse.dve_spec import (Spec as _Spec, Src0 as _Src0, Src1 as _Src1,
                                C0 as _C0, C1 as _C1, Zero as _Zero,
                                lower as _lower, _has_src1)
from concourse.dve_table_gen import dve_ver_for as _dve_ver_for
from concourse.dve_uop import DveOpSpec as _DveOpSpec


def _register_dve(name, spec):
    if name in _dve_ops._SUB_OPCODE_FOR_NAME:
        for o in _dve_ops.OPS:
            if o.name == name:
                return o
    ver = _dve_ver_for("TRN2")
    opcode = max(_dve_ops._SUB_OPCODE_FOR_NAME.values()) + 1
    assert opcode < 0x20
    sha = _DveOpSpec(name=name, opcode=opcode, uops=_lower(spec, ver=ver),
                     rd1_en=_has_src1(spec)).sha(ver)
    dop = _DveOp(name, spec, subdim=False, uops_sha={ver: sha})
    _dve_ops.OPS.append(dop)
    _dve_ops.CUSTOM_DVE_SPECS[name] = spec
    _dve_ops._SUB_OPCODE_FOR_NAME[name] = opcode
    return dop


def _lif_s_ref(in0, in1, s0, s1, imm2):
    P = in1.astype(np.float32) * s0 + in0
    s = (P + s1 >= 0).astype(np.float32)
    return s, s.reshape(s.shape[0], -1).sum(axis=-1, keepdims=True)


def _lif_v_ref(in0, in1, s0, s1, imm2):
    P = in1.astype(np.float32) * s0 + in0
    s = (P + s1 >= 0).astype(np.float32)
    return P + s * s1


# s = ((v*a_mem + isyn) + negThr) >= 0 ; accum = sum(s) over free dim
_P = _Src1 * _C0 + _Src0
LIF_S = _register_dve("LIF_S", _Spec(body=(_P + _C1) >= _Zero,
                                     accum=_op.add, reference=_lif_s_ref))
# v' = P + ((P + negThr) >= 0) * negThr
LIF_V = _register_dve("LIF_V", _Spec(body=_P + ((_P + _C1) >= _Zero) * _C1,
                                     reference=_lif_v_ref))

RS = np.float32(2.0 ** 13)   # residual-plane scale (RESID_K only)


def _build(a_mem, a_syn, lr, tgt):
    """Build + compile the per-core Bass graph (same graph on all 8 cores)."""
    nc = bacc.Bacc("TRN2", target_bir_lowering=False, debug=False,
                   num_devices=NCORES)
    # spikes: flat, per chunk c: KT blocks of [128, B*tc] contiguous, fp8
    spk8 = nc.dram_tensor("spk8", [KT * 128, B * T], mybir.dt.float8e4,
                          kind="ExternalInput").ap()
    # weights: [i128, (k,ht,h)] fp16 -> one contiguous DMA
    wgt = nc.dram_tensor("wgt", [128, KT * 2 * 128], mybir.dt.float16,
                         kind="ExternalInput").ap()
    if RIDER:
        # rider moving pattern delta_{b',b} x ones(TC), one block per
        # distinct TC, concatenated: [64, sum_tc B*tc]
        rid = nc.dram_tensor("rid", [64, B * sum(TCS)], mybir.dt.float8e4,
                             kind="ExternalInput").ap()
        crd = nc.dram_tensor("crd", [64, 2 * 128], mybir.dt.bfloat16,
                             kind="ExternalInput").ap()
    if RESID_K:
        wgt8 = nc.dram_tensor("wgt8", [128, KT * 2 * 128], mybir.dt.float8e4,
                              kind="ExternalInput").ap()
        spkr = nc.dram_tensor("spkr", [KT * 128, B * T // RESID_K],
                              mybir.dt.float8e4, kind="ExternalInput").ap()
    nt0 = nc.dram_tensor("nt0", [128, 2], _F32, kind="ExternalInput").ap()
    odt = mybir.dt.bfloat16
    out = nc.dram_tensor("out", [128, T * 128], odt, kind="ExternalOutput").ap()

    a_mem, a_syn, lr, tgt = float(a_mem), float(a_syn), float(lr), float(tgt)
    c_ema = float(np.float32(-lr / 6400.0))
    k1 = float(np.float32(0.01 * lr * tgt))
    r0 = float(np.float32(lr * tgt))

    with tile.TileContext(nc) as tc:
        with tc.tile_pool(name="wpool", bufs=1) as wpool, \
             tc.tile_pool(name="state", bufs=1) as state, \
             tc.tile_pool(name="spkp", bufs=2) as spkp, \
             tc.tile_pool(name="psum", bufs=2, space="PSUM") as psum, \
             tc.tile_pool(name="wev", bufs=3) as wev, \
             tc.tile_pool(name="accp", bufs=3) as accp, \
             tc.tile_pool(name="tmp", bufs=12) as tmp:

            # ---- persistent tiles ----
            # weights split into 4 k-group tiles, interleaved with the first
            # chunk's spike DMA on the sync queue so the PE can start on
            # k-group 0 while the rest stream in
            KG = 4
            wsbk = [wpool.tile([128, KG * 2 * 128], mybir.dt.float16,
                               tag=f"wsb{g}", name=f"wsb{g}")
                    for g in range(KT // KG)]
            nc.sync.dma_start(wsbk[0][:],
                              wgt[:, 0:KG * 2 * 128])
            if RIDER:
                rsb = wpool.tile([64, B * sum(TCS)], mybir.dt.float8e4,
                                 tag="rsb")
                nc.scalar.dma_start(rsb[:], rid[:])
                roff = {tc_: B * sum(t for t in TCS if t < tc_)
                        for tc_ in TCS}
                csb = wpool.tile([64, 2 * 128], mybir.dt.bfloat16, tag="csb")
                nc.scalar.dma_start(csb[:], crd[:])
            if RESID_K:
                wsb8 = wpool.tile([128, KT * 2 * 128], mybir.dt.float8e4,
                                  tag="wsb8")
                nc.scalar.dma_start(wsb8[:], wgt8[:])
            nT = state.tile([128, 2], _F32, tag="nT")
            nc.scalar.dma_start(nT[:], nt0[:])
            Rst = state.tile([128, 2], _F32, tag="Rst")
            nc.vector.memset(Rst[:], r0)
            K1t = state.tile([128, 2], _F32, tag="K1t")
            nc.vector.memset(K1t[:], k1)
            C99t = state.tile([128, 2], _F32, tag="C99t")
            nc.gpsimd.memset(C99t[:], 0.99)
            Cct = state.tile([128, 2], _F32, tag="Cct")
            nc.gpsimd.memset(Cct[:], c_ema)
            Ust = state.tile([128, 2], _F32, tag="Ust")
            nc.gpsimd.memset(Ust[:], float(np.float32(0.99 * r0 + k1)))
            nTp = state.tile([128, 2], _F32, tag="nTp")
            vst = [state.tile([128, 128], _F32, tag=f"v{i}", name=f"v{i}")
                   for i in range(2)]
            ist = [state.tile([128, 128], _F32, tag=f"i{i}", name=f"isyn{i}")
                   for i in range(4)]
            nc.vector.memset(vst[0][:], 0.0)
            nc.vector.memset(ist[0][:], 0.0)
            # nTp = nT + u, precomputed one step ahead (off critical path)
            nc.gpsimd.tensor_tensor(nTp[:], nT[:], Ust[:], op=_ALU.add)

            t0 = 0
            for c in range(NCH):
                TC = CHUNKS[c]
                cols0 = B * t0          # column offset into per-k row space
                # ---- matmul chunk: weighted[h, (b,t)] for t in chunk ----
                spk_t = spkp.tile([128, KT * B * TC], mybir.dt.float8e4,
                                  tag="spk", name=f"spk_c{c}")
                nc.sync.dma_start(
                    spk_t[:].rearrange("p (k n) -> p k n", k=KT),
                    spk8.rearrange("(k p) n -> p k n",
                                   k=KT)[:, :, cols0:cols0 + B * TC])
                if c == 0:
                    # stream the remaining weight k-groups behind chunk 0
                    for g in range(1, KT // KG):
                        nc.sync.dma_start(
                            wsbk[g][:],
                            wgt[:, g * KG * 2 * 128:(g + 1) * KG * 2 * 128])
                if RESID_K:
                    NR = B * TC // RESID_K
                    spr_t = spkp.tile([128, KT * NR], mybir.dt.float8e4,
                                      tag="spr", name=f"spr_c{c}")
                    for k in range(KT):
                        nc.sync.dma_start(
                            spr_t[:, k * NR:(k + 1) * NR],
                            spkr[k * 128:(k + 1) * 128,
                                 cols0 // RESID_K:cols0 // RESID_K + NR])
                ps = [psum.tile([128, B * TC], _F32, tag=f"ps{ht}",
                                name=f"ps{c}_{ht}") for ht in range(2)]
                blks = _blocks(B * TC)
                for k in range(KT):
                    for ht in range(2):
                        lhsT = wsbk[k // KG][:, ((k % KG) * 2 + ht) * 128:
                                             ((k % KG) * 2 + ht + 1) * 128]
                        for (bo, bn) in blks:
                            nc.tensor.matmul(
                                ps[ht][:, bo:bo + bn],
                                lhsT,
                                spk_t[:, k * B * TC + bo:
                                      k * B * TC + bo + bn],
                                start=(k == 0),
                                stop=False)
                if RESID_K:
                    # residual plane: fp8 DoubleRow over K-step-averaged
                    # packed spikes -> separate PSUM, added during evac
                    NR = B * TC // RESID_K
                    nrblk = max(1, NR // 512)
                    rblk = NR // nrblk
                    pr = [psum.tile([128, NR], _F32, tag=f"pr{ht}",
                                    name=f"pr{c}_{ht}") for ht in range(2)]
                    for kp in range(KT // 2):
                        for ht in range(2):
                            l8 = wsb8[:, ((kp * 2) * 2 + ht) * 128 - 0:
                                      0] if False else None
                            # layout: [i, (kp, ht, ko, h)]
                            l8 = wsb8[:, (kp * 2 * 2 + ht * 2) * 128:
                                      (kp * 2 * 2 + ht * 2 + 2) * 128].rearrange(
                                "p (ko h) -> p ko h", ko=2)
                            r8 = spr_t[:, (2 * kp) * NR:
                                       (2 * kp + 2) * NR].rearrange(
                                "p (ko n) -> p ko n", ko=2)
                            for blk in range(nrblk):
                                nc.tensor.matmul(
                                    pr[ht][:, blk * rblk:(blk + 1) * rblk],
                                    l8, r8[:, :, blk * rblk:(blk + 1) * rblk],
                                    start=(kp == 0), stop=(kp == KT // 2 - 1),
                                    perf_mode=mybir.MatmulPerfMode.DoubleRow)
                assert RIDER  # rider matmuls carry the psum-group stop
                for ht in range(2):
                    lhsT = csb[:, ht * 128:(ht + 1) * 128]
                    for (bo, bn) in blks:
                        nc.tensor.matmul(
                            ps[ht][:, bo:bo + bn],
                            lhsT,
                            rsb[:, roff[TC] + bo:roff[TC] + bo + bn],
                            start=False, stop=True)

                # ---- evacuate PSUM -> SBUF (scalar engine) ----
                wt_ev = wev.tile([128, 2 * B * TC], _F32, tag="wt_ev")
                with tc.high_priority():
                    for ht in range(2):
                        nc.scalar.copy(
                            wt_ev[:, ht * B * TC:(ht + 1) * B * TC],
                            ps[ht][:])
                if RESID_K:
                    # add K-averaged residual (descale by 1/RS) into wt_ev,
                    # broadcast over the K steps of each group (stride-0 ap)
                    w4 = wt_ev[:].rearrange("p (h b g s) -> p h b g s",
                                            h=2, b=B, g=TC // RESID_K)
                    for s in range(RESID_K):
                        for ht in range(2):
                            nc.vector.scalar_tensor_tensor(
                                w4[:, ht, :, :, s],
                                pr[ht][:].rearrange("p (b g) -> p b g", b=B),
                                float(1.0 / RS),
                                w4[:, ht, :, :, s],
                                op0=_ALU.mult, op1=_ALU.add)
                w3 = wt_ev[:].rearrange("p (h b t) -> p h b t", h=2, b=B)

                # ---- LIF scan over this chunk ----
                acc = accp.tile([128, TC * 128], odt, tag="acc")
                for tl in range(TC):
                    t = t0 + tl
                    iold, inew = ist[t % 4], ist[(t + 1) % 4]
                    vold, vnew = vst[t % 2], vst[(t + 1) % 2]
                    i3o = iold[:].rearrange("p (h b) -> p h b", h=2)
                    i3n = inew[:].rearrange("p (h b) -> p h b", h=2)
                    # i_syn = a_syn*i_syn + weighted[t]
                    nc.vector.scalar_tensor_tensor(
                        i3n, i3o, a_syn, w3[:, :, :, tl],
                        op0=_ALU.mult, op1=_ALU.add)
                    rs = tmp.tile([128, 2], _F32, tag="rs")
                    # both LIF_S first (completes rs as early as possible),
                    # then both LIF_V — V doesn't read S's outputs
                    for ht in range(2):
                        sl = slice(ht * B, (ht + 1) * B)
                        s_out = acc[:, tl * 128 + ht * B:
                                    tl * 128 + (ht + 1) * B]
                        # s = ((a_mem*v + i) + nT) >= 0 ; rs = sum_b s
                        nc.vector._custom_dve(
                            LIF_S, out=s_out, in0=inew[:, sl],
                            in1=vold[:, sl],
                            s0=a_mem, s1=nT[:, ht:ht + 1],
                            accum_out=rs[:, ht:ht + 1])
                    for ht in range(2):
                        sl = slice(ht * B, (ht + 1) * B)
                        # v' = P + s*nT
                        nc.vector._custom_dve(
                            LIF_V, out=vnew[:, sl], in0=inew[:, sl],
                            in1=vold[:, sl], s0=a_mem, s1=nT[:, ht:ht + 1])
                    # threshold EMA: the whole critical update is ONE DVE op
                    # (in-order behind V1, no cross-engine wait):
                    #   nT' = nTp + (-lr/6400)*rsum
                    # Pool handles the off-path bookkeeping for the next step:
                    #   t2p = (-lr/6400)*rs ; u' = 0.99*(u + t2p) + k1 ;
                    #   nTp' = nT' + u'
                    nc.vector.scalar_tensor_tensor(
                        nT[:], rs[:], c_ema, nTp[:],
                        op0=_ALU.mult, op1=_ALU.add)
                    t2p = tmp.tile([128, 2], _F32, tag="t2p")
                    nc.gpsimd.tensor_tensor(t2p[:], rs[:], Cct[:],
                                            op=_ALU.mult)
                    nc.gpsimd.tensor_tensor(t2p[:], Ust[:], t2p[:],
                                            op=_ALU.add)
                    nc.gpsimd.tensor_tensor(Ust[:], t2p[:], C99t[:],
                                            op=_ALU.mult)
                    nc.gpsimd.tensor_tensor(Ust[:], Ust[:], K1t[:],
                                            op=_ALU.add)
                    nc.gpsimd.tensor_tensor(nTp[:], nT[:], Ust[:],
                                            op=_ALU.add)
                # ---- chunk output ----
                nc.scalar.dma_start(out[:, t0 * 128:(t0 + TC) * 128], acc[:])
                t0 += TC
    nc.compile()
    return nc


_CACHE = {}


def _get_nc(a_mem, a_syn, lr, tgt):
    key = (RIDER, RESID_K, tuple(CHUNKS), float(a_mem), float(a_syn),
           float(lr), float(tgt))
    if key not in _CACHE:
        _CACHE[key] = _build(a_mem, a_syn, lr, tgt)
    return _CACHE[key]


def kernel(input_spikes, weight, synaptic_strength, threshold,
           tau_mem, tau_syn, target_rate, homeostatic_lr):
    import ml_dtypes
    spikes = np.asarray(input_spikes, dtype=np.float32)
    w_eff = (np.asarray(weight, dtype=np.float32)
             * np.asarray(synaptic_strength, dtype=np.float32))
    thr = np.asarray(threshold, dtype=np.float32)
    tau_m = np.float32(tau_mem)
    tau_s = np.float32(tau_syn)
    tgt = np.float32(target_rate)
    lr = np.float32(homeostatic_lr)
    a_mem = np.float32(np.exp(np.float64(np.float32(-DT) / tau_m)))
    a_syn = np.float32(np.exp(np.float64(np.float32(-DT) / tau_s)))

    nc = _get_nc(a_mem, a_syn, lr, tgt)

    # quantize weights; zero fp16 subnormals so host E matches device exactly
    w16 = w_eff.astype(np.float16)
    w16[np.abs(w16.astype(np.float32)) < np.float32(2.0 ** -14)] = \
        np.float16(0.0)
    E = w16.astype(np.float32) - w_eff                      # [I, H]
    if RESID_K:
        # device adds E8/RS on top of w16; effective weight = w16 + E8/RS
        E8 = (-E * np.float32(RS)).astype(ml_dtypes.float8_e4m3)
        E_used = (w16.astype(np.float32)
                  + E8.astype(np.float32) / np.float32(RS)) - w_eff
    else:
        E_used = E
    sbar = spikes.mean(axis=2)                              # [B, I], exact
    c_full = -(sbar @ E_used)                               # [B, H] fp32

    # spikes [B,I,T] -> [I, B*T] chunk-blocked contiguous, fp8
    sIT = spikes.transpose(1, 0, 2)      # [I, B, T]
    pieces = []
    t0 = 0
    for tc_ in CHUNKS:
        pieces.append(sIT[:, :, t0:t0 + tc_].reshape(I, B * tc_))
        t0 += tc_
    spk_prep = np.ascontiguousarray(np.concatenate(pieces, axis=1))
    spk8_prep = spk_prep.astype(ml_dtypes.float8_e4m3)      # [I, B*T]

    if RESID_K:
        # K-step-averaged packed spikes, same chunk blocking, [I, B*T/K]
        piecesr = []
        t0 = 0
        for tc_ in CHUNKS:
            blk = sIT[:, :, t0:t0 + tc_].reshape(I, B, tc_ // RESID_K,
                                                 RESID_K)
            piecesr.append(blk.mean(axis=3, dtype=np.float32)
                           .reshape(I, B * tc_ // RESID_K))
            t0 += tc_
        spkr_prep = np.ascontiguousarray(
            np.concatenate(piecesr, axis=1)).astype(ml_dtypes.float8_e4m3)

    # rider moving data: delta_{b',b} x ones(tc), one block per distinct TC
    if RIDER:
        piecesd = [np.kron(np.eye(B, dtype=np.float32),
                           np.ones((1, tc_), np.float32))
                   for tc_ in sorted(set(CHUNKS))]
        rid_prep = np.ascontiguousarray(
            np.concatenate(piecesd, axis=1)).astype(ml_dtypes.float8_e4m3)

    in_maps = []
    for core in range(NCORES):
        shard16 = w16[:, core * HL:(core + 1) * HL]          # [I, 256] fp16
        wk = shard16.reshape(KT, 128, 2, 128).transpose(0, 2, 1, 3)
        wk = np.ascontiguousarray(wk.transpose(2, 0, 1, 3)
                                  ).reshape(128, KT * 2 * 128)  # [i,(k,ht,h)]
        nt0 = np.ascontiguousarray(
            -thr[core * HL:(core + 1) * HL].reshape(2, 128).T)
        im = {"wgt": wk, "nt0": nt0, "spk8": spk8_prep}
        if RIDER:
            cs = c_full[:, core * HL:(core + 1) * HL]        # [64, 256]
            im["crd"] = np.ascontiguousarray(
                cs.astype(ml_dtypes.bfloat16))               # [64,(ht,h)]
            im["rid"] = rid_prep
        if RESID_K:
            s8 = E8[:, core * HL:(core + 1) * HL]            # [I, 256]
            # [k,ht,i,h] -> [kp,ko,ht,i,h] -> [i,(kp,ht,ko,h)]
            w8 = s8.reshape(KT // 2, 2, 128, 2, 128).transpose(
                2, 0, 3, 1, 4)
            im["wgt8"] = np.ascontiguousarray(w8).reshape(128, KT * 2 * 128)
            im["spkr"] = spkr_prep
        in_maps.append(im)

    res = run_bass_kernel_spmd(nc, in_maps, core_ids=list(range(NCORES)),
                               trace=TRACE, **TRACE_KW)
    kernel.last_result = res

    outs = []
    for core in range(NCORES):
        o = res.results[core]["out"].astype(np.float32).reshape(128, T, 2, B)
        outs.append(o.transpose(3, 2, 0, 1).reshape(B, HL, T))
    return np.ascontiguousarray(np.concatenate(outs, axis=1))


# revision 30
# speedup vs baseline: 1.0061x; 1.0061x over previous
"""AdaptiveNeuromorphicNetwork Trainium2 kernel (8 NeuronCores, SPMD).

Sharding: output neurons H=2048 split 256/core (H-shard) -> the LIF scan,
spike-rate mean (over batch) and homeostatic threshold update are fully local
per core; zero collectives. input_spikes are replicated (each core streams all
of them through the TensorEngine against its weight column shard).

Matmul scheme ("fp16dc"): single fp16-weight pass with the MOVING operand
being the fp8 spikes directly (0/1 exact in fp8; cost model keys cycles/row
on the moving dtype, so this is the same 1 cyc/row as fp16 moving but needs
no fp16 spike copies/casts at all). The dropped fp8 residual plane is
compensated by a host-computed DC correction: the time-mean residual current
c[b,h] = -sum_i (fp16(w)-w)[i,h] * mean_t s[b,i,t], which the double-EMA
dynamics would otherwise amplify ~4x over the white part. c is injected into
the same PSUM accumulation group as a 64-row "rider" matmul (bf16 weights =
c per batch row, fp8 moving identity-per-batch pattern). CPU-validated rel
err 0.0165 (gate 2e-2); fp16-subnormal weights are zeroed host-side so the
host-side E matches the device bit-exactly.

Optional RESID_K adds a time-averaged fp8-DoubleRow residual plane over
packed spike groups (s_t+...+s_{t+K-1})/K — K=2 halves the white-noise
variance for 0.25 of a pass (est rel err 0.0117, +27us) — off by default.

Per-core pipeline over 9 time-chunks: DMA fp8 spikes -> matmul into PSUM
(16 k-tiles + rider) -> evac PSUM->SBUF (scalar engine) -> sequential LIF
scan (vector engine custom fused DVE ops; threshold EMA chain on gpsimd) ->
spikes accumulate in SBUF (bf16) -> chunked DMA out.
"""
import numpy as np

import concourse.bass as bass
import concourse.tile as tile
from concourse import bacc, mybir
from concourse.bass_utils import run_bass_kernel_spmd

B, I, H, T = 64, 2048, 2048, 128
NCORES = 8
HL = H // NCORES            # 256 output neurons per core
KT = I // 128               # 16 contraction tiles
# tapered at both ends: small first chunks let the PE start before the bulk
# spike DMA lands; small last chunks keep the serial LIF-scan tail short
CHUNKS = [2, 4, 8, 16, 16, 16, 16, 16, 16, 10, 4, 2, 2]
NCH = len(CHUNKS)
assert sum(CHUNKS) == T
DT = 0.001
TCS = sorted(set(CHUNKS))    # distinct chunk lengths (rider pattern per TC)


def _blocks(n):
    """Chop n columns into PSUM-bank-aligned matmul blocks (<=512 f32)."""
    out = []
    o = 0
    while o < n:
        b = min(512, n - o)
        out.append((o, b))
        o += b
    return out

RIDER = True        # inject DC correction via 64-row rider matmul
RESID_K = 0         # 0=off; 2/4 = K-step-averaged fp8 residual plane
TRACE = False
TRACE_KW = {}

_F32 = mybir.dt.float32
_ALU = mybir.AluOpType

# ---- custom fused DVE ops for the LIF step ----
import operator as _op

import concourse.dve_ops as _dve_ops
from concourse.dve_ops import DveOp as _DveOp
from concourse.dve_spec import (Spec as _Spec, Src0 as _Src0, Src1 as _Src1,
                                C0 as _C0, C1 as _C1, Zero as _Zero,
                                lower as _lower, _has_src1)
from concourse.dve_table_gen import dve_ver_for as _dve_ver_for
from concourse.dve_uop import DveOpSpec as _DveOpSpec


def _register_dve(name, spec):
    if name in _dve_ops._SUB_OPCODE_FOR_NAME:
        for o in _dve_ops.OPS:
            if o.name == name:
                return o
    ver = _dve_ver_for("TRN2")
    opcode = max(_dve_ops._SUB_OPCODE_FOR_NAME.values()) + 1
    assert opcode < 0x20
    sha = _DveOpSpec(name=name, opcode=opcode, uops=_lower(spec, ver=ver),
                     rd1_en=_has_src1(spec)).sha(ver)
    dop = _DveOp(name, spec, subdim=False, uops_sha={ver: sha})
    _dve_ops.OPS.append(dop)
    _dve_ops.CUSTOM_DVE_SPECS[name] = spec
    _dve_ops._SUB_OPCODE_FOR_NAME[name] = opcode
    return dop


def _lif_s_ref(in0, in1, s0, s1, imm2):
    P = in1.astype(np.float32) * s0 + in0
    s = (P + s1 >= 0).astype(np.float32)
    return s, s.reshape(s.shape[0], -1).sum(axis=-1, keepdims=True)


def _lif_v_ref(in0, in1, s0, s1, imm2):
    P = in1.astype(np.float32) * s0 + in0
    s = (P + s1 >= 0).astype(np.float32)
    return P + s * s1


# s = ((v*a_mem + isyn) + negThr) >= 0 ; accum = sum(s) over free dim
_P = _Src1 * _C0 + _Src0
LIF_S = _register_dve("LIF_S", _Spec(body=(_P + _C1) >= _Zero,
                                     accum=_op.add, reference=_lif_s_ref))
# v' = P + ((P + negThr) >= 0) * negThr
LIF_V = _register_dve("LIF_V", _Spec(body=_P + ((_P + _C1) >= _Zero) * _C1,
                                     reference=_lif_v_ref))

RS = np.float32(2.0 ** 13)   # residual-plane scale (RESID_K only)


def _build(a_mem, a_syn, lr, tgt):
    """Build + compile the per-core Bass graph (same graph on all 8 cores)."""
    nc = bacc.Bacc("TRN2", target_bir_lowering=False, debug=False,
                   num_devices=NCORES)
    # spikes: flat, per chunk c: KT blocks of [128, B*tc] contiguous, fp8
    spk8 = nc.dram_tensor("spk8", [KT * 128, B * T], mybir.dt.float8e4,
                          kind="ExternalInput").ap()
    # weights: [i128, (k,ht,h)] fp16 -> one contiguous DMA
    wgt = nc.dram_tensor("wgt", [128, KT * 2 * 128], mybir.dt.float16,
                         kind="ExternalInput").ap()
    if RIDER:
        # rider moving pattern delta_{b',b} x ones(TC), one block per
        # distinct TC, concatenated: [64, sum_tc B*tc]
        rid = nc.dram_tensor("rid", [64, B * sum(TCS)], mybir.dt.float8e4,
                             kind="ExternalInput").ap()
        crd = nc.dram_tensor("crd", [64, 2 * 128], mybir.dt.bfloat16,
                             kind="ExternalInput").ap()
    if RESID_K:
        wgt8 = nc.dram_tensor("wgt8", [128, KT * 2 * 128], mybir.dt.float8e4,
                              kind="ExternalInput").ap()
        spkr = nc.dram_tensor("spkr", [KT * 128, B * T // RESID_K],
                              mybir.dt.float8e4, kind="ExternalInput").ap()
    nt0 = nc.dram_tensor("nt0", [128, 2], _F32, kind="ExternalInput").ap()
    odt = mybir.dt.bfloat16
    out = nc.dram_tensor("out", [128, T * 128], odt, kind="ExternalOutput").ap()

    a_mem, a_syn, lr, tgt = float(a_mem), float(a_syn), float(lr), float(tgt)
    c_ema = float(np.float32(-lr / 6400.0))
    k1 = float(np.float32(0.01 * lr * tgt))
    r0 = float(np.float32(lr * tgt))

    with tile.TileContext(nc) as tc:
        with tc.tile_pool(name="wpool", bufs=1) as wpool, \
             tc.tile_pool(name="state", bufs=1) as state, \
             tc.tile_pool(name="spkp", bufs=2) as spkp, \
             tc.tile_pool(name="psum", bufs=2, space="PSUM") as psum, \
             tc.tile_pool(name="wev", bufs=3) as wev, \
             tc.tile_pool(name="accp", bufs=3) as accp, \
             tc.tile_pool(name="tmp", bufs=12) as tmp:

            # ---- persistent tiles ----
            # weights split into 4 k-group tiles, interleaved with the first
            # chunk's spike DMA on the sync queue so the PE can start on
            # k-group 0 while the rest stream in
            KG = 4
            wsbk = [wpool.tile([128, KG * 2 * 128], mybir.dt.float16,
                               tag=f"wsb{g}", name=f"wsb{g}")
                    for g in range(KT // KG)]
            nc.sync.dma_start(wsbk[0][:],
                              wgt[:, 0:KG * 2 * 128])
            if RIDER:
                rsb = wpool.tile([64, B * sum(TCS)], mybir.dt.float8e4,
                                 tag="rsb")
                nc.scalar.dma_start(rsb[:], rid[:])
                roff = {tc_: B * sum(t for t in TCS if t < tc_)
                        for tc_ in TCS}
                csb = wpool.tile([64, 2 * 128], mybir.dt.bfloat16, tag="csb")
                nc.scalar.dma_start(csb[:], crd[:])
            if RESID_K:
                wsb8 = wpool.tile([128, KT * 2 * 128], mybir.dt.float8e4,
                                  tag="wsb8")
                nc.scalar.dma_start(wsb8[:], wgt8[:])
            nT = state.tile([128, 2], _F32, tag="nT")
            nc.scalar.dma_start(nT[:], nt0[:])
            Rst = state.tile([128, 2], _F32, tag="Rst")
            nc.vector.memset(Rst[:], r0)
            K1t = state.tile([128, 2], _F32, tag="K1t")
            nc.vector.memset(K1t[:], k1)
            C99t = state.tile([128, 2], _F32, tag="C99t")
            nc.gpsimd.memset(C99t[:], 0.99)
            Cct = state.tile([128, 2], _F32, tag="Cct")
            nc.gpsimd.memset(Cct[:], c_ema)
            Ust = state.tile([128, 2], _F32, tag="Ust")
            nc.gpsimd.memset(Ust[:], float(np.float32(0.99 * r0 + k1)))
            nTp = state.tile([128, 2], _F32, tag="nTp")
            vst = [state.tile([128, 128], _F32, tag=f"v{i}", name=f"v{i}")
                   for i in range(2)]
            ist = [state.tile([128, 128], _F32, tag=f"i{i}", name=f"isyn{i}")
                   for i in range(4)]
            nc.vector.memset(vst[0][:], 0.0)
            nc.vector.memset(ist[0][:], 0.0)
            # nTp = nT + u, precomputed one step ahead (off critical path)
            nc.gpsimd.tensor_tensor(nTp[:], nT[:], Ust[:], op=_ALU.add)

            t0 = 0
            for c in range(NCH):
                TC = CHUNKS[c]
                cols0 = B * t0          # column offset into per-k row space
                # ---- matmul chunk: weighted[h, (b,t)] for t in chunk ----
                spk_t = spkp.tile([128, KT * B * TC], mybir.dt.float8e4,
                                  tag="spk", name=f"spk_c{c}")
                nc.sync.dma_start(
                    spk_t[:].rearrange("p (k n) -> p k n", k=KT),
                    spk8.rearrange("(k p) n -> p k n",
                                   k=KT)[:, :, cols0:cols0 + B * TC])
                if c == 0:
                    # stream the remaining weight k-groups behind chunk 0
                    for g in range(1, KT // KG):
                        nc.sync.dma_start(
                            wsbk[g][:],
                            wgt[:, g * KG * 2 * 128:(g + 1) * KG * 2 * 128])
                if RESID_K:
                    NR = B * TC // RESID_K
                    spr_t = spkp.tile([128, KT * NR], mybir.dt.float8e4,
                                      tag="spr", name=f"spr_c{c}")
                    for k in range(KT):
                        nc.sync.dma_start(
                            spr_t[:, k * NR:(k + 1) * NR],
                            spkr[k * 128:(k + 1) * 128,
                                 cols0 // RESID_K:cols0 // RESID_K + NR])
                ps = [psum.tile([128, B * TC], _F32, tag=f"ps{ht}",
                                name=f"ps{c}_{ht}") for ht in range(2)]
                blks = _blocks(B * TC)
                for k in range(KT):
                    for ht in range(2):
                        lhsT = wsbk[k // KG][:, ((k % KG) * 2 + ht) * 128:
                                             ((k % KG) * 2 + ht + 1) * 128]
                        for (bo, bn) in blks:
                            nc.tensor.matmul(
                                ps[ht][:, bo:bo + bn],
                                lhsT,
                                spk_t[:, k * B * TC + bo:
                                      k * B * TC + bo + bn],
                                start=(k == 0),
                                stop=False)
                if RESID_K:
                    # residual plane: fp8 DoubleRow over K-step-averaged
                    # packed spikes -> separate PSUM, added during evac
                    NR = B * TC // RESID_K
                    nrblk = max(1, NR // 512)
                    rblk = NR // nrblk
                    pr = [psum.tile([128, NR], _F32, tag=f"pr{ht}",
                                    name=f"pr{c}_{ht}") for ht in range(2)]
                    for kp in range(KT // 2):
                        for ht in range(2):
                            l8 = wsb8[:, ((kp * 2) * 2 + ht) * 128 - 0:
                                      0] if False else None
                            # layout: [i, (kp, ht, ko, h)]
                            l8 = wsb8[:, (kp * 2 * 2 + ht * 2) * 128:
                                      (kp * 2 * 2 + ht * 2 + 2) * 128].rearrange(
                                "p (ko h) -> p ko h", ko=2)
                            r8 = spr_t[:, (2 * kp) * NR:
                                       (2 * kp + 2) * NR].rearrange(
                                "p (ko n) -> p ko n", ko=2)
                            for blk in range(nrblk):
                                nc.tensor.matmul(
                                    pr[ht][:, blk * rblk:(blk + 1) * rblk],
                                    l8, r8[:, :, blk * rblk:(blk + 1) * rblk],
                                    start=(kp == 0), stop=(kp == KT // 2 - 1),
                                    perf_mode=mybir.MatmulPerfMode.DoubleRow)
                assert RIDER  # rider matmuls carry the psum-group stop
                for ht in range(2):
                    lhsT = csb[:, ht * 128:(ht + 1) * 128]
                    for (bo, bn) in blks:
                        nc.tensor.matmul(
                            ps[ht][:, bo:bo + bn],
                            lhsT,
                            rsb[:, roff[TC] + bo:roff[TC] + bo + bn],
                            start=False, stop=True)

                # ---- evacuate PSUM -> SBUF (scalar engine) ----
                # split into two time-halves with separate tiles so the scan
                # unblocks after the first half instead of the whole chunk
                TCH = max(1, TC // 2)
                wt_h = [wev.tile([128, 2 * B * TCH], _F32, tag=f"wt{h}",
                                 name=f"wt{c}_{h}")
                        for h in range(2 if TC > 1 else 1)]
                with tc.high_priority():
                    for h in range(len(wt_h)):
                        dst = wt_h[h][:].rearrange("p (g b t) -> p g b t",
                                                   g=2, b=B)
                        for ht in range(2):
                            src = ps[ht][:].rearrange(
                                "p (b t) -> p b t",
                                b=B)[:, :, h * TCH:(h + 1) * TCH]
                            nc.scalar.copy(dst[:, ht], src)
                assert not RESID_K, "RESID_K unsupported with split evac"
                w3h = [t_[:].rearrange("p (g b t) -> p g b t", g=2, b=B)
                       for t_ in wt_h]

                # ---- LIF scan over this chunk ----
                acc = accp.tile([128, TC * 128], odt, tag="acc")
                for tl in range(TC):
                    t = t0 + tl
                    iold, inew = ist[t % 4], ist[(t + 1) % 4]
                    vold, vnew = vst[t % 2], vst[(t + 1) % 2]
                    i3o = iold[:].rearrange("p (h b) -> p h b", h=2)
                    i3n = inew[:].rearrange("p (h b) -> p h b", h=2)
                    # i_syn = a_syn*i_syn + weighted[t]
                    nc.vector.scalar_tensor_tensor(
                        i3n, i3o, a_syn,
                        w3h[min(tl // TCH, len(w3h) - 1)][:, :, :, tl % TCH],
                        op0=_ALU.mult, op1=_ALU.add)
                    rs = tmp.tile([128, 2], _F32, tag="rs")
                    # both LIF_S first (completes rs as early as possible),
                    # then both LIF_V — V doesn't read S's outputs
                    for ht in range(2):
                        sl = slice(ht * B, (ht + 1) * B)
                        s_out = acc[:, tl * 128 + ht * B:
                                    tl * 128 + (ht + 1) * B]
                        # s = ((a_mem*v + i) + nT) >= 0 ; rs = sum_b s
                        nc.vector._custom_dve(
                            LIF_S, out=s_out, in0=inew[:, sl],
                            in1=vold[:, sl],
                            s0=a_mem, s1=nT[:, ht:ht + 1],
                            accum_out=rs[:, ht:ht + 1])
                    for ht in range(2):
                        sl = slice(ht * B, (ht + 1) * B)
                        # v' = P + s*nT
                        nc.vector._custom_dve(
                            LIF_V, out=vnew[:, sl], in0=inew[:, sl],
                            in1=vold[:, sl], s0=a_mem, s1=nT[:, ht:ht + 1])
                    # threshold EMA: the whole critical update is ONE DVE op
                    # (in-order behind V1, no cross-engine wait):
                    #   nT' = nTp + (-lr/6400)*rsum
                    # Pool handles the off-path bookkeeping for the next step:
                    #   t2p = (-lr/6400)*rs ; u' = 0.99*(u + t2p) + k1 ;
                    #   nTp' = nT' + u'
                    nc.vector.scalar_tensor_tensor(
                        nT[:], rs[:], c_ema, nTp[:],
                        op0=_ALU.mult, op1=_ALU.add)
                    t2p = tmp.tile([128, 2], _F32, tag="t2p")
                    nc.gpsimd.tensor_tensor(t2p[:], rs[:], Cct[:],
                                            op=_ALU.mult)
                    nc.gpsimd.tensor_tensor(t2p[:], Ust[:], t2p[:],
                                            op=_ALU.add)
                    nc.gpsimd.tensor_tensor(Ust[:], t2p[:], C99t[:],
                                            op=_ALU.mult)
                    nc.gpsimd.tensor_tensor(Ust[:], Ust[:], K1t[:],
                                            op=_ALU.add)
                    nc.gpsimd.tensor_tensor(nTp[:], nT[:], Ust[:],
                                            op=_ALU.add)
                # ---- chunk output ----
                nc.scalar.dma_start(out[:, t0 * 128:(t0 + TC) * 128], acc[:])
                t0 += TC
    nc.compile()
    return nc


_CACHE = {}


def _get_nc(a_mem, a_syn, lr, tgt):
    key = (RIDER, RESID_K, tuple(CHUNKS), float(a_mem), float(a_syn),
           float(lr), float(tgt))
    if key not in _CACHE:
        _CACHE[key] = _build(a_mem, a_syn, lr, tgt)
    return _CACHE[key]


def kernel(input_spikes, weight, synaptic_strength, threshold,
           tau_mem, tau_syn, target_rate, homeostatic_lr):
    import ml_dtypes
    spikes = np.asarray(input_spikes, dtype=np.float32)
    w_eff = (np.asarray(weight, dtype=np.float32)
             * np.asarray(synaptic_strength, dtype=np.float32))
    thr = np.asarray(threshold, dtype=np.float32)
    tau_m = np.float32(tau_mem)
    tau_s = np.float32(tau_syn)
    tgt = np.float32(target_rate)
    lr = np.float32(homeostatic_lr)
    a_mem = np.float32(np.exp(np.float64(np.float32(-DT) / tau_m)))
    a_syn = np.float32(np.exp(np.float64(np.float32(-DT) / tau_s)))

    nc = _get_nc(a_mem, a_syn, lr, tgt)

    # quantize weights; zero fp16 subnormals so host E matches device exactly
    w16 = w_eff.astype(np.float16)
    w16[np.abs(w16.astype(np.float32)) < np.float32(2.0 ** -14)] = \
        np.float16(0.0)
    E = w16.astype(np.float32) - w_eff                      # [I, H]
    if RESID_K:
        # device adds E8/RS on top of w16; effective weight = w16 + E8/RS
        E8 = (-E * np.float32(RS)).astype(ml_dtypes.float8_e4m3)
        E_used = (w16.astype(np.float32)
                  + E8.astype(np.float32) / np.float32(RS)) - w_eff
    else:
        E_used = E
    sbar = spikes.mean(axis=2)                              # [B, I], exact
    c_full = -(sbar @ E_used)                               # [B, H] fp32

    # spikes [B,I,T] -> [I, B*T] chunk-blocked contiguous, fp8
    sIT = spikes.transpose(1, 0, 2)      # [I, B, T]
    pieces = []
    t0 = 0
    for tc_ in CHUNKS:
        pieces.append(sIT[:, :, t0:t0 + tc_].reshape(I, B * tc_))
        t0 += tc_
    spk_prep = np.ascontiguousarray(np.concatenate(pieces, axis=1))
    spk8_prep = spk_prep.astype(ml_dtypes.float8_e4m3)      # [I, B*T]

    if RESID_K:
        # K-step-averaged packed spikes, same chunk blocking, [I, B*T/K]
        piecesr = []
        t0 = 0
        for tc_ in CHUNKS:
            blk = sIT[:, :, t0:t0 + tc_].reshape(I, B, tc_ // RESID_K,
                                                 RESID_K)
            piecesr.append(blk.mean(axis=3, dtype=np.float32)
                           .reshape(I, B * tc_ // RESID_K))
            t0 += tc_
        spkr_prep = np.ascontiguousarray(
            np.concatenate(piecesr, axis=1)).astype(ml_dtypes.float8_e4m3)

    # rider moving data: delta_{b',b} x ones(tc), one block per distinct TC
    if RIDER:
        piecesd = [np.kron(np.eye(B, dtype=np.float32),
                           np.ones((1, tc_), np.float32))
                   for tc_ in sorted(set(CHUNKS))]
        rid_prep = np.ascontiguousarray(
            np.concatenate(piecesd, axis=1)).astype(ml_dtypes.float8_e4m3)

    in_maps = []
    for core in range(NCORES):
        shard16 = w16[:, core * HL:(core + 1) * HL]          # [I, 256] fp16
        wk = shard16.reshape(KT, 128, 2, 128).transpose(0, 2, 1, 3)
        wk = np.ascontiguousarray(wk.transpose(2, 0, 1, 3)
                                  ).reshape(128, KT * 2 * 128)  # [i,(k,ht,h)]
        nt0 = np.ascontiguousarray(
            -thr[core * HL:(core + 1) * HL].reshape(2, 128).T)
        im = {"wgt": wk, "nt0": nt0, "spk8": spk8_prep}
        if RIDER:
            cs = c_full[:, core * HL:(core + 1) * HL]        # [64, 256]
            im["crd"] = np.ascontiguousarray(
                cs.astype(ml_dtypes.bfloat16))               # [64,(ht,h)]
            im["rid"] = rid_prep
        if RESID_K:
            s8 = E8[:, core * HL:(core + 1) * HL]            # [I, 256]
            # [k,ht,i,h] -> [kp,ko,ht,i,h] -> [i,(kp,ht,ko,h)]
            w8 = s8.reshape(KT // 2, 2, 128, 2, 128).transpose(
                2, 0, 3, 1, 4)
            im["wgt8"] = np.ascontiguousarray(w8).reshape(128, KT * 2 * 128)
            im["spkr"] = spkr_prep
        in_maps.append(im)

    res = run_bass_kernel_spmd(nc, in_maps, core_ids=list(range(NCORES)),
                               trace=TRACE, **TRACE_KW)
    kernel.last_result = res

    outs = []
    for core in range(NCORES):
        o = res.results[core]["out"].astype(np.float32).reshape(128, T, 2, B)
        outs.append(o.transpose(3, 2, 0, 1).reshape(B, HL, T))
    return np.ascontiguousarray(np.concatenate(outs, axis=1))
